# revision 26
# baseline (speedup 1.0000x reference)
"""Trainium2 Bass kernel for an attention layer whose math collapses.

The module computes softmax over a size-1 axis, so the attention weights
are exactly 1.0 and the output is context[b, 0, d] = sum_t a[b, t, d].
The MLP branch (W1, b1, W2, b2) and s_prev never affect the output.

Strategy: pure data parallel over the batch axis; each of the 8 cores
reduces its [16, 512, 512] shard over the time axis. Memory-bound:
~16 MiB HBM read per core (~38 us window at ~440 GB/s aggregate over
both HWDGE rings).

Kernel shape (per core):
  - The 16 MiB shard is loaded as 16 slabs of 1 MiB (one batch each),
    DMA'd as [128 partitions x 8 KiB contiguous] (large descriptors,
    all 16 SDMA engines engaged). Even slabs go on the SP HWDGE ring,
    odd slabs on the Activation ring, so per-DMA fixed costs overlap
    and slabs arrive every ~2.4 us.
  - Each slab holds one batch: 4 time-rows of 512 per partition.
    Measured engine rates: fp32 PE matmul is ~1.2 us per 512 cols
    (HI/LO split; streaming everything through the PE costs 75 us),
    DVE tensor_reduce is 1x-mode with a stride penalty. Fastest is 2
    contiguous in-place halving adds per slab (2048 -> 1024 -> 512,
    ~1.9 us on DVE, ~2x that on GPSIMD). Early slabs fold on GPSIMD,
    the rest on the faster DVE, so both keep up with arrivals and the
    last slab folds fast. Chained same-engine adds need a semaphore
    handshake (deep pipelines have no RAW interlock).
  - One fp32 matmul per slab against the preamble's constant ones
    [128, 1] vector reduces across partitions into a psum row. Eight
    psum banks hold 2 slab results each at partition offsets {0, 32}
    (PE output base partition is limited to {0, 32, 64}).
  - ACT bounces each psum row to SBUF; per-slab 2 KiB stores overlap
    all but the last store's latency.

Raw Bass (not Tile): the HW allows very few sync-waits per instruction,
which fights Tile's auto-generated waits; with per-DMA completion
semaphores every wait is a standalone single-condition instruction and
Tile's tail barriers are avoided.
"""

from contextlib import ExitStack

import numpy as np

B, TX, D = 128, 512, 512
N_CORES = 8
NB = B // N_CORES   # 16 batches per core
P = 128             # SBUF partitions
NSLAB = 16          # 1 MiB DMA slabs per core (= one batch per slab)
FPP = NB * TX * D // (NSLAB * P)  # f32 per partition per slab = 2048

# Slabs folded on GPSIMD (early arrivals; ~2x slower than DVE) vs DVE.
POOL_SLABS = (0, 1, 2, 3)

_CACHE: dict = {}


def _build_bass():
    import concourse.bass as bass
    import concourse.mybir as mybir

    f32 = mybir.dt.float32
    add = mybir.AluOpType.add
    nc = bass.Bass("TRN2")
    a = nc.dram_tensor("a", [NB, TX, D], f32, kind="ExternalInput")
    out = nc.dram_tensor("out", [NB, D], f32, kind="ExternalOutput")

    ones = nc.const_aps.aps[(f32, 1.0)]  # preamble-initialized [128, 1]
    a_sl = a.rearrange("b t d -> (b t d)").rearrange(
        "(g p f) -> g p f", g=NSLAB, p=P
    )

    with ExitStack() as ctx:
        abuf = ctx.enter_context(nc.sbuf_tensor([P, NSLAB * FPP], f32))
        ost = ctx.enter_context(nc.sbuf_tensor([1, NB * D], f32))
        psb = [
            ctx.enter_context(nc.psum_tensor(f"ps{i}", [64, D], f32))
            for i in range(8)
        ]
        # One completion semaphore per DMA: concurrent DMA completions
        # are unordered, so a shared counting sem would be racy.
        ld_sems = [
            ctx.enter_context(nc.semaphore(f"ld_sem{g}")) for g in range(NSLAB)
        ]
        fold_sems = [
            ctx.enter_context(nc.semaphore(f"fold_sem{g}")) for g in range(NSLAB)
        ]
        red_sems = [
            ctx.enter_context(nc.semaphore(f"red_sem{g}")) for g in range(NSLAB)
        ]
        st_sems = [
            ctx.enter_context(nc.semaphore(f"st_sem{g}")) for g in range(NSLAB)
        ]
        pe_sem = ctx.enter_context(nc.semaphore("pe_sem"))
        cp_sem = ctx.enter_context(nc.semaphore("cp_sem"))
        # Skip Block's exit all-engine barrier (several us of event-
        # semaphore ping-pong): every DMA/engine is already quiesced by
        # the explicit final waits, and the preamble re-clears semaphores
        # on each run. Class-level patch: Bass resolves the method on the
        # class, so an instance attribute would not take effect.
        orig_barrier = bass.Bass.all_engine_barrier
        bass.Bass.all_engine_barrier = lambda self, *a, **k: None
        ctx.callback(
            lambda: setattr(bass.Bass, "all_engine_barrier", orig_barrier)
        )
        block = ctx.enter_context(nc.Block(no_gpsimd_drain=True))

        abuf_t = abuf[:].rearrange("p (g f) -> p g f", g=NSLAB)

        def fold_slab(eng, g):
            """2 in-place contiguous halving adds: 2048 -> 512 f32/partition.
            Same-engine RAW needs an explicit sem handshake per step."""
            eng.wait_ge(ld_sems[g], 16)
            sl = abuf_t[:, g]
            h = FPP // 2
            eng.tensor_tensor(sl[:, 0:h], sl[:, 0:h], sl[:, h : 2 * h], add).then_inc(
                fold_sems[g], 1
            )
            eng.wait_ge(fold_sems[g], 1)
            h = FPP // 4
            eng.tensor_tensor(sl[:, 0:h], sl[:, 0:h], sl[:, h : 2 * h], add).then_inc(
                red_sems[g], 1
            )

        @block.sync
        def _(sync):
            for g in range(0, NSLAB, 2):
                sync.dma_start(out=abuf_t[:, g], in_=a_sl[g]).then_inc(ld_sems[g], 16)
            # Per-slab 2 KiB stores: all but the last store's latency
            # overlaps with remaining compute.
            for g in range(NSLAB):
                sync.wait_ge(cp_sem, g + 1)
                sync.dma_start(
                    out=out[g : g + 1, :], in_=ost[0:1, g * D : (g + 1) * D]
                ).then_inc(st_sems[g], 16)
            for g in range(NSLAB):
                sync.wait_ge(st_sems[g], 16)

        @block.scalar
        def _(scalar):
            # Second HWDGE ring (Activation sequencer) for the odd slabs.
            for g in range(1, NSLAB, 2):
                scalar.dma_start(out=abuf_t[:, g], in_=a_sl[g]).then_inc(
                    ld_sems[g], 16
                )
            # ACT also bounces finished psum rows to SBUF (DMA cannot
            # read PSUM; DVE/GPSIMD are busy folding slabs).
            for g in range(NSLAB):
                off = 32 * (g % 2)
                scalar.wait_ge(pe_sem, g + 1)
                scalar.copy(
                    ost[:, g * D : (g + 1) * D], psb[g // 2][off : off + 1, :]
                ).then_inc(cp_sem, 1)

        @block.gpsimd
        def _(gpsimd):
            for g in POOL_SLABS:
                fold_slab(gpsimd, g)

        @block.vector
        def _(vector):
            for g in range(NSLAB):
                if g not in POOL_SLABS:
                    fold_slab(vector, g)

        @block.tensor
        def _(tensor):
            for g in range(NSLAB):
                off = 32 * (g % 2)
                tensor.wait_ge(red_sems[g], 1)
                tensor.matmul(
                    psb[g // 2][off : off + 1, :],
                    lhsT=ones[:, 0:1],
                    rhs=abuf_t[:, g, 0:D],
                    start=True,
                    stop=True,
                ).then_inc(pe_sem, 1)

    return nc


def _get_bass():
    if "nc" not in _CACHE:
        _CACHE["nc"] = _build_bass()
    return _CACHE["nc"]


def run_spmd(a, **spmd_kwargs):
    """Run the SPMD kernel on all 8 cores; returns (full_output, BassKernelResults)."""
    from concourse.bass_utils import run_bass_kernel_spmd

    nc = _get_bass()
    a = np.ascontiguousarray(np.asarray(a), dtype=np.float32)
    assert a.shape == (B, TX, D), a.shape
    in_maps = [{"a": a[k * NB : (k + 1) * NB]} for k in range(N_CORES)]
    res = run_bass_kernel_spmd(nc, in_maps, list(range(N_CORES)), **spmd_kwargs)
    out = np.concatenate([res.results[k]["out"] for k in range(N_CORES)], axis=0)
    return out.reshape(B, 1, D).astype(np.float32), res


def kernel(a, s_prev=None, W1=None, b1=None, W2=None, b2=None, **_unused):
    out, _ = run_spmd(a)
    return out


# revision 27
# speedup vs baseline: 1.1129x; 1.1129x over previous
"""Trainium2 Bass kernel for an attention layer whose math collapses.

The module computes softmax over a size-1 axis, so the attention weights
are exactly 1.0 and the output is context[b, 0, d] = sum_t a[b, t, d].
The MLP branch (W1, b1, W2, b2) and s_prev never affect the output.

Strategy: pure data parallel over the batch axis; each of the 8 cores
reduces its [16, 512, 512] shard over the time axis. Memory-bound:
~16 MiB HBM read per core (~38 us window at ~440 GB/s aggregate over
both HWDGE rings).

Kernel shape (per core):
  - The 16 MiB shard is loaded as 16 slabs of 1 MiB (one batch each),
    DMA'd as [128 partitions x 8 KiB contiguous] (large descriptors,
    all 16 SDMA engines engaged). Even slabs go on the SP HWDGE ring,
    odd slabs on the Activation ring, so per-DMA fixed costs overlap
    and slabs arrive every ~2.4 us.
  - Each slab holds one batch: 4 time-rows of 512 per partition.
    Measured engine rates: fp32 PE matmul is ~1.2 us per 512 cols
    (HI/LO split; streaming everything through the PE costs 75 us),
    DVE tensor_reduce is 1x-mode with a stride penalty. Fastest is 2
    contiguous in-place halving adds per slab (2048 -> 1024 -> 512,
    ~1.9 us on DVE, ~2x that on GPSIMD). Early slabs fold on GPSIMD,
    the rest on the faster DVE, so both keep up with arrivals and the
    last slab folds fast. Chained same-engine adds need a semaphore
    handshake (deep pipelines have no RAW interlock).
  - One fp32 matmul per slab against the preamble's constant ones
    [128, 1] vector reduces across partitions into a psum row. Eight
    psum banks hold 2 slab results each at partition offsets {0, 32}
    (PE output base partition is limited to {0, 32, 64}).
  - ACT bounces each psum row to SBUF; per-slab 2 KiB stores overlap
    all but the last store's latency.

Raw Bass (not Tile): the HW allows very few sync-waits per instruction,
which fights Tile's auto-generated waits; with per-DMA completion
semaphores every wait is a standalone single-condition instruction and
Tile's tail barriers are avoided.
"""

from contextlib import ExitStack

import numpy as np

B, TX, D = 128, 512, 512
N_CORES = 8
NB = B // N_CORES   # 16 batches per core
P = 128             # SBUF partitions
NSLAB = 16          # 1 MiB DMA slabs per core (= one batch per slab)
FPP = NB * TX * D // (NSLAB * P)  # f32 per partition per slab = 2048

# Slabs folded on GPSIMD (early arrivals; ~2x slower than DVE) vs DVE.
POOL_SLABS = (0, 1, 2, 3)

_CACHE: dict = {}


def _build_bass():
    import concourse.bass as bass
    import concourse.mybir as mybir

    f32 = mybir.dt.float32
    add = mybir.AluOpType.add
    nc = bass.Bass("TRN2")
    a = nc.dram_tensor("a", [NB, TX, D], f32, kind="ExternalInput")
    out = nc.dram_tensor("out", [NB, D], f32, kind="ExternalOutput")

    ones = nc.const_aps.aps[(f32, 1.0)]  # preamble-initialized [128, 1]
    a_sl = a.rearrange("b t d -> (b t d)").rearrange(
        "(g p f) -> g p f", g=NSLAB, p=P
    )

    with ExitStack() as ctx:
        abuf = ctx.enter_context(nc.sbuf_tensor([P, NSLAB * FPP], f32))
        ost = ctx.enter_context(nc.sbuf_tensor([1, NB * D], f32))
        psb = [
            ctx.enter_context(nc.psum_tensor(f"ps{i}", [64, D], f32))
            for i in range(8)
        ]
        # One completion semaphore per DMA: concurrent DMA completions
        # are unordered, so a shared counting sem would be racy.
        ld_sems = [
            ctx.enter_context(nc.semaphore(f"ld_sem{g}")) for g in range(NSLAB)
        ]
        fold_sems = [
            ctx.enter_context(nc.semaphore(f"fold_sem{g}")) for g in range(NSLAB)
        ]
        red_sems = [
            ctx.enter_context(nc.semaphore(f"red_sem{g}")) for g in range(NSLAB)
        ]
        st_sems = [
            ctx.enter_context(nc.semaphore(f"st_sem{g}")) for g in range(NSLAB)
        ]
        pe_sem = ctx.enter_context(nc.semaphore("pe_sem"))
        cp_sem = ctx.enter_context(nc.semaphore("cp_sem"))
        block = ctx.enter_context(nc.Block(no_gpsimd_drain=True))

        abuf_t = abuf[:].rearrange("p (g f) -> p g f", g=NSLAB)

        def fold_slab(eng, g):
            """2 in-place contiguous halving adds: 2048 -> 512 f32/partition.
            Same-engine RAW needs an explicit sem handshake per step."""
            eng.wait_ge(ld_sems[g], 16)
            sl = abuf_t[:, g]
            h = FPP // 2
            eng.tensor_tensor(sl[:, 0:h], sl[:, 0:h], sl[:, h : 2 * h], add).then_inc(
                fold_sems[g], 1
            )
            eng.wait_ge(fold_sems[g], 1)
            h = FPP // 4
            eng.tensor_tensor(sl[:, 0:h], sl[:, 0:h], sl[:, h : 2 * h], add).then_inc(
                red_sems[g], 1
            )

        @block.sync
        def _(sync):
            for g in range(0, NSLAB, 2):
                sync.dma_start(out=abuf_t[:, g], in_=a_sl[g]).then_inc(ld_sems[g], 16)
            # Per-slab 2 KiB stores: all but the last store's latency
            # overlaps with remaining compute.
            for g in range(NSLAB):
                sync.wait_ge(cp_sem, g + 1)
                sync.dma_start(
                    out=out[g : g + 1, :], in_=ost[0:1, g * D : (g + 1) * D]
                ).then_inc(st_sems[g], 16)
            for g in range(NSLAB):
                sync.wait_ge(st_sems[g], 16)

        @block.scalar
        def _(scalar):
            # Second HWDGE ring (Activation sequencer) for the odd slabs.
            for g in range(1, NSLAB, 2):
                scalar.dma_start(out=abuf_t[:, g], in_=a_sl[g]).then_inc(
                    ld_sems[g], 16
                )
            # ACT also bounces finished psum rows to SBUF (DMA cannot
            # read PSUM; DVE/GPSIMD are busy folding slabs).
            for g in range(NSLAB):
                off = 32 * (g % 2)
                scalar.wait_ge(pe_sem, g + 1)
                scalar.copy(
                    ost[:, g * D : (g + 1) * D], psb[g // 2][off : off + 1, :]
                ).then_inc(cp_sem, 1)

        @block.gpsimd
        def _(gpsimd):
            for g in POOL_SLABS:
                fold_slab(gpsimd, g)

        @block.vector
        def _(vector):
            for g in range(NSLAB):
                if g not in POOL_SLABS:
                    fold_slab(vector, g)

        @block.tensor
        def _(tensor):
            for g in range(NSLAB):
                off = 32 * (g % 2)
                tensor.wait_ge(red_sems[g], 1)
                tensor.matmul(
                    psb[g // 2][off : off + 1, :],
                    lhsT=ones[:, 0:1],
                    rhs=abuf_t[:, g, 0:D],
                    start=True,
                    stop=True,
                ).then_inc(pe_sem, 1)

    return nc


def _get_bass():
    if "nc" not in _CACHE:
        _CACHE["nc"] = _build_bass()
    return _CACHE["nc"]


def run_spmd(a, **spmd_kwargs):
    """Run the SPMD kernel on all 8 cores; returns (full_output, BassKernelResults)."""
    from concourse.bass_utils import run_bass_kernel_spmd

    nc = _get_bass()
    a = np.ascontiguousarray(np.asarray(a), dtype=np.float32)
    assert a.shape == (B, TX, D), a.shape
    in_maps = [{"a": a[k * NB : (k + 1) * NB]} for k in range(N_CORES)]
    res = run_bass_kernel_spmd(nc, in_maps, list(range(N_CORES)), **spmd_kwargs)
    out = np.concatenate([res.results[k]["out"] for k in range(N_CORES)], axis=0)
    return out.reshape(B, 1, D).astype(np.float32), res


def kernel(a, s_prev=None, W1=None, b1=None, W2=None, b2=None, **_unused):
    out, _ = run_spmd(a)
    return out
